# revision 3
# baseline (speedup 1.0000x reference)
"""Trainium2 Bass kernel for nn_BinarizedLinear:
    out = sign(input_b @ sign(weight).T)
input_b (8192, 2048), weight (2048, 2048), entries +/-1.0 fp32.

Data-parallel across 8 NeuronCores: each core takes 1024 rows of input_b,
full weight replicated.  Per-core HBM traffic (24MB in + 8MB out) at
~360GB/s sets an ~90us floor; the kernel tracks that DMA wall:
  - SWDGE (gpsimd) DMA casts fp32->fp8e4 in flight - no separate cast
    pass, fp8 staging slabs only.  The first slabs' DMAs issue before any
    other gpsimd work so bytes flow from ~2us.
  - adjacent fp8 k-pairs are viewed as fp16 and PE-transposed as 16-bit
    elements: 8 transposes per slab instead of 16.  (The only fp8 values
    are +/-1.0, so every byte-pair is a normal fp16 value and passes the
    PE datapath bit-exactly.)
  - DoubleRow fp8 matmuls with contraction (p,t): k = 256q + 2p + t.
    lhsT (stationary xt) is de-interleaved at eviction (Ldweights requires
    contiguous operands); the moving wt operand streams a strided fp8 view
    of the packed fp16 tile directly.
  - strict engine separation so no queue head-of-line blocks production:
    ACT does evictions only, DVE does signs only, sync issues out DMAs,
    gpsimd issues input DMAs.
  - HAM: the PE clock gate halves the rate (k=4/8) unless real matmuls
    flow; transposes don't count.  Dummy fp8-DR bursts run between early
    production units (through the supply-limited phase) to hold k=8.
Production closes o-blocks early (all W done with 3 x slabs to come) so
the backlog drains under the last x DMAs and the post-DMA tail is one
bt's worth of matmuls.
"""

import numpy as np

BATCH, IN_LEN, OUT_LEN = 8192, 2048, 2048
N_CORES = 8
SHARD = BATCH // N_CORES  # 1024
P = 128

_cache = {}


def build_kernel(shard=SHARD, in_len=IN_LEN, out_len=OUT_LEN,
                 warm0=8, warm_group=2, warm_units=16, warm_intra=1,
                 mm_lag_flush=3, pre_issue=3, production=None):
    import concourse.mybir as mybir
    import concourse.tile as tile
    from concourse import bacc
    from concourse.masks import make_identity

    f32 = mybir.dt.float32
    fp8 = mybir.dt.float8e4
    f16 = mybir.dt.float16

    KT = in_len // P          # 16 k-tiles of 128 (fp8 view)
    KQ = KT // 2              # 8 packed k-groups (fp16 view)
    BT = shard // P           # 8 b-tiles per core
    OB = out_len // 512       # 4 o-blocks of 512
    WS = out_len // P         # 16 W slabs of 128 rows
    IW = in_len // 2          # fp16 elements per slab row (1024)

    nc = bacc.Bacc(None, target_bir_lowering=False)
    x = nc.dram_tensor("x", [shard, in_len], f32, kind="ExternalInput")
    w = nc.dram_tensor("w", [out_len, in_len], f32, kind="ExternalInput")
    out = nc.dram_tensor("out", [shard, out_len], f16, kind="ExternalOutput")
    scratch = nc.dram_tensor("scratch", [1, 1], f32, kind="ExternalOutput")

    # Production order: ob0 closes after 4 units for early matmuls; all W
    # done with 3 x slabs to come; x-only tail.
    if production is None:
        production = [("w", 0), ("w", 1), ("w", 2), ("w", 3),
                      ("x", 0),
                      ("w", 4), ("w", 5), ("w", 6), ("w", 7),
                      ("x", 1),
                      ("w", 8), ("w", 9), ("x", 2),
                      ("w", 10), ("w", 11), ("x", 3),
                      ("w", 12), ("w", 13), ("x", 4),
                      ("w", 14), ("w", 15),
                      ("x", 5), ("x", 6), ("x", 7)]
    assert sorted(s for k, s in production if k == "w") == list(range(WS))
    assert sorted(s for k, s in production if k == "x") == list(range(BT))

    with tile.TileContext(nc) as tc:
        with (
            tc.tile_pool(name="const", bufs=1) as const_pool,
            tc.tile_pool(name="stage", bufs=10) as stage_pool,
            tc.tile_pool(name="xt", bufs=BT) as xt_pool,
            tc.tile_pool(name="wt", bufs=OB) as wt_pool,
            tc.tile_pool(name="outs", bufs=16) as out_pool,
            tc.tile_pool(name="tpsum", bufs=2, space="PSUM") as tpsum_pool,
            tc.tile_pool(name="mpsum", bufs=5, space="PSUM") as mpsum_pool,
            tc.tile_pool(name="wpsum", bufs=1, space="PSUM") as wpsum_pool,
        ):
            def emit_slab_dma(item):
                """Issue the SWDGE cast DMAs for one slab; return staging."""
                kind, s = item
                dram, row0 = (x, s * P) if kind == "x" else (w, s * P)
                half = in_len // 2
                s8 = stage_pool.tile([P, in_len], fp8, tag="s8")
                nc.gpsimd.dma_start(out=s8[:, :half],
                                    in_=dram[row0:row0 + P, :half])
                nc.gpsimd.dma_start(out=s8[:, half:],
                                    in_=dram[row0:row0 + P, half:])
                return s8

            # head: first slabs' DMAs go first on the gpsimd queue
            staged = {}
            for item in production[:pre_issue]:
                staged[item] = emit_slab_dma(item)

            # HAM warmup + constants (gpsimd queue work sits after the
            # pre-issued DMAs; PE warmup needs no inputs)
            warm_src = const_pool.tile([P, 2, 512], fp8, name="warm_src")
            nc.vector.memset(warm_src[:], 1.0)
            warm_psum = wpsum_pool.tile([P, 512], f32, name="warm_psum")

            def warm_burst(n):
                for i in range(n):
                    nc.tensor.matmul(
                        warm_psum[:], warm_src[:, :, :P], warm_src[:],
                        start=(i == 0), stop=(i == n - 1),
                        perf_mode=mybir.MatmulPerfMode.DoubleRow,
                    )

            warm_burst(warm0)

            def warm_flush():
                warm_out = const_pool.tile([1, 1], f32, name="warm_out")
                nc.vector.tensor_copy(out=warm_out[:], in_=warm_psum[:1, :1])
                nc.sync.dma_start(out=scratch[:], in_=warm_out[:])

            ident = const_pool.tile([P, P], f16)
            make_identity(nc, ident)

            # resident operands; contraction pair (p,t) -> k = 256q + 2p + t.
            xt = {bt: xt_pool.tile([P, KQ, 2, P], fp8, tag="xt", name=f"xt{bt}")
                  for bt in range(BT)}
            wt = {ob: wt_pool.tile([P, KQ, 2, 512], fp8, tag="wt",
                                   name=f"wt{ob}") for ob in range(OB)}
            wt8 = {ob: wt[ob][:] for ob in range(OB)}

            def emit_slab_compute(item, s8, warm_mid=0):
                """PE-transpose 8 fp16 pair tiles into one PSUM bank, evict
                to the resident tile (ACT only).  warm_mid inserts dummy DR
                matmuls mid-slab so the HAM activity window never empties
                during transpose-only stretches (transposes don't count)."""
                kind, s = item
                s16 = s8[:].bitcast(f16)  # [P, IW]
                tp = tpsum_pool.tile([P, IW], f16, tag="tp")
                for q in range(KQ):
                    if warm_mid and q == KQ // 2:
                        warm_burst(warm_mid)
                    nc.tensor.transpose(
                        tp[:, q * P:(q + 1) * P],
                        s16[:, q * P:(q + 1) * P],
                        ident[:],
                    )
                if kind == "x":
                    bt = s
                    nc.vector.tensor_copy(
                        out=xt[bt][:],
                        in_=tp[:].bitcast(fp8).rearrange(
                            "p (q b t) -> p q t b", q=KQ, t=2),
                    )
                else:
                    ob, j = s // 4, s % 4
                    nc.scalar.copy(
                        out=wt[ob][:, :, :, j * P:(j + 1) * P],
                        in_=tp[:].bitcast(fp8).rearrange(
                            "p (q o t) -> p q t o", q=KQ, t=2),
                    )

            def produce(item, warm_mid=0):
                s8 = staged.pop(item, None)
                if s8 is None:
                    s8 = emit_slab_dma(item)
                emit_slab_compute(item, s8, warm_mid)

            def emit_mm(ob, bt):
                psum = mpsum_pool.tile([P, 512], f32)
                for q in range(KQ):
                    nc.tensor.matmul(
                        psum[:], xt[bt][:, q], wt8[ob][:, q],
                        start=(q == 0), stop=(q == KQ - 1),
                        perf_mode=mybir.MatmulPerfMode.DoubleRow,
                    )
                ot = out_pool.tile([P, 512], f16)
                nc.vector.tensor_scalar(
                    out=ot[:], in0=psum[:], scalar1=1.0, scalar2=-1.0,
                    op0=mybir.AluOpType.min, op1=mybir.AluOpType.max,
                )
                nc.sync.dma_start(
                    out=out[bt * P:(bt + 1) * P, ob * 512:(ob + 1) * 512],
                    in_=ot[:],
                )

            x_done, w_done = set(), set()
            x_unit, w_unit = {}, {}
            mm_todo = [(ob, bt) for ob in range(OB) for bt in range(BT)]

            def flush_mms(limit, xset, wset):
                ready = [(ob, bt) for (ob, bt) in mm_todo
                         if ob in wset and bt in xset]
                ready.sort(key=lambda p: max(w_unit.get(p[0], 0),
                                             x_unit.get(p[1], 0)))
                for ob, bt in ready[:limit]:
                    emit_mm(ob, bt)
                    mm_todo.remove((ob, bt))

            late_u = len(production) - 4
            prev_x, prev_w = set(), set()
            for u, item in enumerate(production):
                if 0 < u <= warm_units:
                    warm_burst(warm_group)
                if u == warm_units + 1:
                    warm_flush()
                wm = warm_intra if 0 < u <= warm_units else 0
                if item[0] == "x":
                    produce(item, wm)
                    x_done.add(item[1])
                    x_unit[item[1]] = u
                else:
                    produce(item, wm)
                    if item[1] % 4 == 3:
                        w_done.add(item[1] // 4)
                        w_unit[item[1] // 4] = u
                flush_mms(mm_lag_flush if u < late_u else 6, prev_x, prev_w)
                prev_x, prev_w = set(x_done), set(w_done)
            if warm_units + 1 >= len(production):
                warm_flush()
            while mm_todo:
                flush_mms(4, x_done, w_done)
                if mm_todo:
                    warm_burst(1)

    nc.finalize()
    return nc


def _get_nc():
    if "nc" not in _cache:
        _cache["nc"] = build_kernel()
    return _cache["nc"]


def run_sharded(input_b, weight, trace=False):
    """Run the SPMD kernel; returns (output, BassKernelResults)."""
    from concourse.bass_utils import run_bass_kernel_spmd

    nc = _get_nc()
    input_b = np.ascontiguousarray(input_b, dtype=np.float32)
    weight = np.ascontiguousarray(weight, dtype=np.float32)
    in_maps = [
        {"x": input_b[c * SHARD:(c + 1) * SHARD], "w": weight}
        for c in range(N_CORES)
    ]
    res = run_bass_kernel_spmd(nc, in_maps, list(range(N_CORES)), trace=trace)
    # device writes the sign values (exactly -1/0/+1) as fp16; upcast on
    # the host during the gather - a pure re-encoding, values identical
    out = np.concatenate(
        [np.asarray(res.results[c]["out"]).astype(np.float32)
         for c in range(N_CORES)], axis=0)
    return out, res


def kernel(input_b, weight):
    out, _ = run_sharded(input_b, weight, trace=False)
    return out


# revision 4
# speedup vs baseline: 1.1163x; 1.1163x over previous
"""Trainium2 Bass kernel for nn_BinarizedLinear:
    out = sign(input_b @ sign(weight).T)
input_b (8192, 2048), weight (2048, 2048), entries +/-1.0 fp32.

Data-parallel across 8 NeuronCores: each core takes 1024 rows of input_b,
full weight replicated.  Per-core HBM traffic (24MB in + 8MB out) at
~360GB/s sets an ~90us floor; the kernel tracks that DMA wall:
  - SWDGE (gpsimd) DMA casts fp32->fp8e4 in flight - no separate cast
    pass, fp8 staging slabs only.  The first slabs' DMAs issue before any
    other gpsimd work so bytes flow from ~2us.
  - adjacent fp8 k-pairs are viewed as fp16 and PE-transposed as 16-bit
    elements: 8 transposes per slab instead of 16.  (The only fp8 values
    are +/-1.0, so every byte-pair is a normal fp16 value and passes the
    PE datapath bit-exactly.)
  - DoubleRow fp8 matmuls with contraction (p,t): k = 256q + 2p + t.
    lhsT (stationary xt) is de-interleaved at eviction (Ldweights requires
    contiguous operands); the moving wt operand streams a strided fp8 view
    of the packed fp16 tile directly.
  - strict engine separation so no queue head-of-line blocks production:
    ACT does evictions only, DVE does signs only, sync issues out DMAs,
    gpsimd issues input DMAs.
  - HAM: the PE clock gate halves the rate (k=4/8) unless real matmuls
    flow; transposes don't count.  Dummy fp8-DR bursts run between early
    production units (through the supply-limited phase) to hold k=8.
Production closes o-blocks early (all W done with 3 x slabs to come) so
the backlog drains under the last x DMAs and the post-DMA tail is one
bt's worth of matmuls.
"""

import numpy as np

BATCH, IN_LEN, OUT_LEN = 8192, 2048, 2048
N_CORES = 8
SHARD = BATCH // N_CORES  # 1024
P = 128

_cache = {}


def build_kernel(shard=SHARD, in_len=IN_LEN, out_len=OUT_LEN,
                 warm0=8, warm_group=2, warm_units=16, warm_intra=1,
                 mm_lag_flush=3, pre_issue=3, production=None):
    import concourse.mybir as mybir
    import concourse.tile as tile
    from concourse import bacc
    from concourse.masks import make_identity

    f32 = mybir.dt.float32
    fp8 = mybir.dt.float8e4
    f16 = mybir.dt.float16

    KT = in_len // P          # 16 k-tiles of 128 (fp8 view)
    KQ = KT // 2              # 8 packed k-groups (fp16 view)
    BT = shard // P           # 8 b-tiles per core
    OB = out_len // 512       # 4 o-blocks of 512
    WS = out_len // P         # 16 W slabs of 128 rows
    IW = in_len // 2          # fp16 elements per slab row (1024)

    nc = bacc.Bacc(None, target_bir_lowering=False)
    x = nc.dram_tensor("x", [shard, in_len], f32, kind="ExternalInput")
    w = nc.dram_tensor("w", [out_len, in_len], f32, kind="ExternalInput")
    out = nc.dram_tensor("out", [shard, out_len], f16, kind="ExternalOutput")
    scratch = nc.dram_tensor("scratch", [1, 1], f32, kind="ExternalOutput")

    # Production order: ob0 closes after 4 units for early matmuls; all W
    # done with 3 x slabs to come; x-only tail.
    if production is None:
        production = [("w", 0), ("w", 1), ("w", 2), ("w", 3),
                      ("x", 0),
                      ("w", 4), ("w", 5), ("w", 6), ("w", 7),
                      ("x", 1),
                      ("w", 8), ("w", 9), ("x", 2),
                      ("w", 10), ("w", 11), ("x", 3),
                      ("w", 12), ("w", 13), ("x", 4),
                      ("w", 14), ("w", 15),
                      ("x", 5), ("x", 6), ("x", 7)]
    assert sorted(s for k, s in production if k == "w") == list(range(WS))
    assert sorted(s for k, s in production if k == "x") == list(range(BT))

    with tile.TileContext(nc) as tc:
        with (
            tc.tile_pool(name="const", bufs=1) as const_pool,
            tc.tile_pool(name="stage", bufs=10) as stage_pool,
            tc.tile_pool(name="xt", bufs=BT) as xt_pool,
            tc.tile_pool(name="wt", bufs=OB) as wt_pool,
            tc.tile_pool(name="outs", bufs=16) as out_pool,
            tc.tile_pool(name="tpsum", bufs=2, space="PSUM") as tpsum_pool,
            tc.tile_pool(name="mpsum", bufs=5, space="PSUM") as mpsum_pool,
            tc.tile_pool(name="wpsum", bufs=1, space="PSUM") as wpsum_pool,
        ):
            def emit_slab_dma(item):
                """Issue the SWDGE cast DMAs for one slab; return staging."""
                kind, s = item
                dram, row0 = (x, s * P) if kind == "x" else (w, s * P)
                half = in_len // 2
                s8 = stage_pool.tile([P, in_len], fp8, tag="s8")
                nc.gpsimd.dma_start(out=s8[:, :half],
                                    in_=dram[row0:row0 + P, :half])
                nc.gpsimd.dma_start(out=s8[:, half:],
                                    in_=dram[row0:row0 + P, half:])
                return s8

            # head: first slabs' DMAs go first on the gpsimd queue
            staged = {}
            for item in production[:pre_issue]:
                staged[item] = emit_slab_dma(item)

            # HAM warmup + constants (gpsimd queue work sits after the
            # pre-issued DMAs; PE warmup needs no inputs)
            warm_src = const_pool.tile([P, 2, 512], fp8, name="warm_src")
            nc.vector.memset(warm_src[:], 1.0)
            warm_psum = wpsum_pool.tile([P, 512], f32, name="warm_psum")

            def warm_burst(n):
                for i in range(n):
                    nc.tensor.matmul(
                        warm_psum[:], warm_src[:, :, :P], warm_src[:],
                        start=(i == 0), stop=(i == n - 1),
                        perf_mode=mybir.MatmulPerfMode.DoubleRow,
                    )

            warm_burst(warm0)

            def warm_flush():
                warm_out = const_pool.tile([1, 1], f32, name="warm_out")
                nc.vector.tensor_copy(out=warm_out[:], in_=warm_psum[:1, :1])
                # scratch goes on the SWDGE queue: a 4-byte DMA mixed with
                # the wide out DMAs on the sync HWDGE queue is the exact
                # narrow/wide fanout mix behind the completion-order hazard
                nc.gpsimd.dma_start(out=scratch[:], in_=warm_out[:])

            ident = const_pool.tile([P, P], f16)
            make_identity(nc, ident)

            # resident operands; contraction pair (p,t) -> k = 256q + 2p + t.
            xt = {bt: xt_pool.tile([P, KQ, 2, P], fp8, tag="xt", name=f"xt{bt}")
                  for bt in range(BT)}
            wt = {ob: wt_pool.tile([P, KQ, 2, 512], fp8, tag="wt",
                                   name=f"wt{ob}") for ob in range(OB)}
            wt8 = {ob: wt[ob][:] for ob in range(OB)}

            def emit_slab_compute(item, s8, warm_mid=0):
                """PE-transpose 8 fp16 pair tiles into one PSUM bank, evict
                to the resident tile (ACT only).  warm_mid inserts dummy DR
                matmuls mid-slab so the HAM activity window never empties
                during transpose-only stretches (transposes don't count)."""
                kind, s = item
                s16 = s8[:].bitcast(f16)  # [P, IW]
                tp = tpsum_pool.tile([P, IW], f16, tag="tp")
                for q in range(KQ):
                    if warm_mid and q == KQ // 2:
                        warm_burst(warm_mid)
                    nc.tensor.transpose(
                        tp[:, q * P:(q + 1) * P],
                        s16[:, q * P:(q + 1) * P],
                        ident[:],
                    )
                if kind == "x":
                    bt = s
                    nc.vector.tensor_copy(
                        out=xt[bt][:],
                        in_=tp[:].bitcast(fp8).rearrange(
                            "p (q b t) -> p q t b", q=KQ, t=2),
                    )
                else:
                    ob, j = s // 4, s % 4
                    nc.scalar.copy(
                        out=wt[ob][:, :, :, j * P:(j + 1) * P],
                        in_=tp[:].bitcast(fp8).rearrange(
                            "p (q o t) -> p q t o", q=KQ, t=2),
                    )

            def produce(item, warm_mid=0):
                s8 = staged.pop(item, None)
                if s8 is None:
                    s8 = emit_slab_dma(item)
                emit_slab_compute(item, s8, warm_mid)

            def emit_mm(ob, bt):
                psum = mpsum_pool.tile([P, 512], f32)
                for q in range(KQ):
                    nc.tensor.matmul(
                        psum[:], xt[bt][:, q], wt8[ob][:, q],
                        start=(q == 0), stop=(q == KQ - 1),
                        perf_mode=mybir.MatmulPerfMode.DoubleRow,
                    )
                ot = out_pool.tile([P, 512], f16)
                nc.vector.tensor_scalar(
                    out=ot[:], in0=psum[:], scalar1=1.0, scalar2=-1.0,
                    op0=mybir.AluOpType.min, op1=mybir.AluOpType.max,
                )
                nc.sync.dma_start(
                    out=out[bt * P:(bt + 1) * P, ob * 512:(ob + 1) * 512],
                    in_=ot[:],
                )

            x_done, w_done = set(), set()
            x_unit, w_unit = {}, {}
            mm_todo = [(ob, bt) for ob in range(OB) for bt in range(BT)]

            def flush_mms(limit, xset, wset):
                ready = [(ob, bt) for (ob, bt) in mm_todo
                         if ob in wset and bt in xset]
                ready.sort(key=lambda p: max(w_unit.get(p[0], 0),
                                             x_unit.get(p[1], 0)))
                for ob, bt in ready[:limit]:
                    emit_mm(ob, bt)
                    mm_todo.remove((ob, bt))

            late_u = len(production) - 4
            prev_x, prev_w = set(), set()
            for u, item in enumerate(production):
                if 0 < u <= warm_units:
                    warm_burst(warm_group)
                if u == warm_units + 1:
                    warm_flush()
                wm = warm_intra if 0 < u <= warm_units else 0
                if item[0] == "x":
                    produce(item, wm)
                    x_done.add(item[1])
                    x_unit[item[1]] = u
                else:
                    produce(item, wm)
                    if item[1] % 4 == 3:
                        w_done.add(item[1] // 4)
                        w_unit[item[1] // 4] = u
                flush_mms(mm_lag_flush if u < late_u else 6, prev_x, prev_w)
                prev_x, prev_w = set(x_done), set(w_done)
            if warm_units + 1 >= len(production):
                warm_flush()
            while mm_todo:
                flush_mms(4, x_done, w_done)
                if mm_todo:
                    warm_burst(1)

    nc.finalize()
    return nc


def _get_nc():
    if "nc" not in _cache:
        _cache["nc"] = build_kernel()
    return _cache["nc"]


def run_sharded(input_b, weight, trace=False):
    """Run the SPMD kernel; returns (output, BassKernelResults)."""
    from concourse.bass_utils import run_bass_kernel_spmd

    nc = _get_nc()
    input_b = np.ascontiguousarray(input_b, dtype=np.float32)
    weight = np.ascontiguousarray(weight, dtype=np.float32)
    in_maps = [
        {"x": input_b[c * SHARD:(c + 1) * SHARD], "w": weight}
        for c in range(N_CORES)
    ]
    res = run_bass_kernel_spmd(nc, in_maps, list(range(N_CORES)), trace=trace)
    # device writes the sign values (exactly -1/0/+1) as fp16; upcast on
    # the host during the gather - a pure re-encoding, values identical
    out = np.concatenate(
        [np.asarray(res.results[c]["out"]).astype(np.float32)
         for c in range(N_CORES)], axis=0)
    return out, res


def kernel(input_b, weight):
    out, _ = run_sharded(input_b, weight, trace=False)
    return out
